# revision 11
# baseline (speedup 1.0000x reference)
"""Trainium2 Bass kernel for nn_Conv2d_24833500905755 (3x3 conv, B=32,
C_in=64, C_out=128, 56x56, pad 1, with the reference's mismatched
weight-flatten order).

Math: out[b,co,h,w] = sum_{c,di,dj} xpad[b,c,h+di,w+dj] * Wt[c,di*3+dj,co]
with Wt = K.reshape(576, C_OUT).reshape(C_IN, 9, C_OUT).

Data-parallel: 4 images per NeuronCore, 2 images packed on the
128-partition dim (fp16 matmuls, K=64 contraction per half, concurrent
PE row-group tiles). Raw-bass hand-scheduled engine programs with
manual semaphores.

DMA plan: the three DMA paths (gpsimd SWDGE ring, Sync HWDGE ring,
Scalar HWDGE ring, each in-order at ~160-190GB/s, ~1.7-2.5us startup)
are specialized so the first-chunk gates (w and x rows 0-10) land as
early as possible:
  gpsimd ring: w (294KB) only                       -> ~9.3us
  Sync ring:   x pair-0 pieces, then half-0 outputs -> x0A ~9.6us
  Scalar ring: x pair-1 pieces, then half-1 outputs
Tensor: 11 junk warmup matmuls (HAM p-state ramp + DMA-latency cover -
any tensor idle gap resets the PE clock ramp and costs ~3us of
half-clock stream, so warmups must bridge seamlessly into the real
stream), then 252 fp16 matmuls gated on input-piece / bank-WAR sems.
Vector: 14 PSUM->SBUF copies (half 0); Scalar: 14 copies (half 1).

Outputs are staged and DMA'd as fp16 (halves HBM write traffic vs
fp32); the host casts back to fp32. PSUM accumulation stays fp32, so
this only adds ~2^-11 relative rounding on the final values.
"""

from contextlib import ExitStack

import numpy as np

import concourse.bass as bass
import concourse.mybir as mybir
from concourse.bass_utils import run_bass_kernel_spmd

B, C_IN, C_OUT, H = 32, 64, 128, 56
KS = 3
N_CORES = 8
BPC = B // N_CORES
HP = H + 2
RCHUNK = 8
NCHUNK = H // RCHUNK          # 7 chunks/image, 14 global chunks (2 pairs)
OBLOCKS = [(0, 24), (24, 40), (40, 48), (48, 56)]
# x row pieces: piece j covers rows [XSPLITS[j], XSPLITS[j+1]).
# chunk ci needs rows [8ci, 8ci+9]: c0<-p0, c1,c2<-p1, c3,c4<-p2, c5,c6<-p3
XSPLITS = [0, 11, 25, 42, HP]
# chunk index -> number of pieces that must have landed
CHUNK_PIECE = [1, 2, 2, 3, 3, 4, 4]
MM_DT = mybir.dt.float16
N_WARMUP = 19
# per-global-chunk row sub-ranges; the very last chunk is split so its
# first copy + output DMA overlap the final matmuls
SUBCHUNKS = {c: [(0, RCHUNK)] for c in range(2 * NCHUNK)}
SUBCHUNKS[2 * NCHUNK - 1] = [(0, 4), (4, RCHUNK)]
# (chunk, sub_r0) -> 1-based completion index in the s_mm/s_cp counting
SUBIDX = {}
_n = 0
for _c in range(2 * NCHUNK):
    for _r0, _r1 in SUBCHUNKS[_c]:
        _n += 1
        SUBIDX[(_c, _r0)] = _n


def build_nc(mm_dt=MM_DT):
    nc = bass.Bass()
    x_ext = nc.declare_dram_parameter("x", [BPC, C_IN, HP, HP], mm_dt, isOutput=False)
    w_ext = nc.declare_dram_parameter("w", [2 * C_IN, KS * KS, C_OUT], mm_dt, isOutput=False)
    out_ext = nc.declare_dram_parameter("out", [BPC, C_OUT, H, H], mm_dt, isOutput=True)

    n_out_dmas = 2 * len(OBLOCKS) * 2  # pairs * blocks * halves

    with ExitStack() as ctx:
        wt = ctx.enter_context(nc.sbuf_tensor("wt", [2 * C_IN, KS * KS, C_OUT], mm_dt))
        xps = [
            ctx.enter_context(nc.sbuf_tensor(f"xp{p}", [2 * C_IN, HP, HP], mm_dt))
            for p in range(2)
        ]
        # obs[p][half][block]
        obs = [
            [
                [
                    ctx.enter_context(
                        nc.sbuf_tensor(f"ob_{p}_{h}_{bi}", [C_OUT, bhi - blo, H], mm_dt)
                    )
                    for bi, (blo, bhi) in enumerate(OBLOCKS)
                ]
                for h in range(2)
            ]
            for p in range(2)
        ]
        # banks[slot][half] - 8 PSUM banks
        banks = [
            [
                ctx.enter_context(
                    nc.psum_tensor(f"ps_{s}_{h}", [C_OUT, RCHUNK, H], mybir.dt.float32)
                )
                for h in range(2)
            ]
            for s in range(4)
        ]
        s_w = ctx.enter_context(nc.semaphore("s_w"))
        s_x = [ctx.enter_context(nc.semaphore(f"s_x{p}")) for p in range(2)]
        s_mm = ctx.enter_context(nc.semaphore("s_mm"))
        s_cp = ctx.enter_context(nc.semaphore("s_cp"))
        s_cp2 = ctx.enter_context(nc.semaphore("s_cp2"))
        s_out = ctx.enter_context(nc.semaphore("s_out"))

        with nc.Block() as block:

            @block.sync
            def _(sync: bass.BassEngine):
                src = x_ext[0:2].rearrange("b c h w -> (b c) h w")
                for j in range(len(XSPLITS) - 1):
                    lo, hi = XSPLITS[j], XSPLITS[j + 1]
                    sync.dma_start(
                        out=xps[0][:, lo:hi, :], in_=src[:, lo:hi, :]
                    ).then_inc(s_x[0], 16)
                for p in range(2):
                    for bi, (blo, bhi) in enumerate(OBLOCKS):
                        c_last = p * NCHUNK + (bhi // RCHUNK - 1)
                        dst = out_ext[2 * p : 2 * p + 1].rearrange("b c h w -> (b c) h w")
                        for r0, r1 in SUBCHUNKS[c_last]:
                            h0 = (bhi // RCHUNK - 1) * RCHUNK
                            sync.wait_ge(s_cp, SUBIDX[(c_last, r0)])
                            dlo = blo if len(SUBCHUNKS[c_last]) == 1 else h0 + r0
                            sync.dma_start(
                                out=dst[:, dlo : h0 + r1, :],
                                in_=obs[p][0][bi][:, dlo - blo : h0 + r1 - blo, :],
                            ).then_inc(s_out, 16)

            @block.scalar
            def _(scalar: bass.BassEngine):
                scalar.dma_start(out=wt[:], in_=w_ext[:]).then_inc(s_w, 16)
                src = x_ext[2:4].rearrange("b c h w -> (b c) h w")
                for j in range(len(XSPLITS) - 1):
                    lo, hi = XSPLITS[j], XSPLITS[j + 1]
                    scalar.dma_start(
                        out=xps[1][:, lo:hi, :], in_=src[:, lo:hi, :]
                    ).then_inc(s_x[1], 16)
                for p in range(2):
                    for ci in range(NCHUNK):
                        c = p * NCHUNK + ci
                        h0 = ci * RCHUNK
                        blo, bhi = next(b for b in OBLOCKS if b[0] <= h0 < b[1])
                        bi = OBLOCKS.index((blo, bhi))
                        for r0, r1 in SUBCHUNKS[c]:
                            slot = c % 4 if r0 == 0 else (c + 1) % 4
                            scalar.wait_ge(s_mm, SUBIDX[(c, r0)])
                            scalar.copy(
                                out=obs[p][1][bi][
                                    :, h0 - blo + r0 : h0 - blo + r1, :
                                ],
                                in_=banks[slot][1][:, 0 : r1 - r0, :],
                            ).then_inc(s_cp2, 1)
                            if h0 + r1 == bhi or len(SUBCHUNKS[c]) > 1:
                                scalar.wait_ge(s_cp2, SUBIDX[(c, r0)])
                                dst = out_ext[2 * p + 1 : 2 * p + 2].rearrange(
                                    "b c h w -> (b c) h w"
                                )
                                dlo = blo if len(SUBCHUNKS[c]) == 1 else h0 + r0
                                scalar.dma_start(
                                    out=dst[:, dlo : h0 + r1, :],
                                    in_=obs[p][1][bi][
                                        :, dlo - blo : h0 + r1 - blo, :
                                    ],
                                ).then_inc(s_out, 16)

            @block.tensor
            def _(tensor: bass.BassEngine):
                # HAM warm-up: junk matmuls on not-yet-loaded SBUF while the
                # first input DMAs land; keeps the PE ramping toward 8/8
                # clock for the real stream. banks[3] is first reused by
                # chunk 3 (start=True clears it), well after these complete.
                for wi in range(N_WARMUP):
                    h = wi % 2
                    c0 = h * C_IN
                    tensor.matmul(
                        out=banks[3][h][:],
                        lhsT=wt[c0 : c0 + C_IN, 0, :],
                        rhs=xps[0][c0 : c0 + C_IN, 0:RCHUNK, 0:H],
                        start=True,
                        stop=True,
                    )
                tensor.wait_ge(s_w, 16)
                for p in range(2):
                    for ci in range(NCHUNK):
                        c = p * NCHUNK + ci
                        h0 = ci * RCHUNK
                        need = CHUNK_PIECE[ci]
                        if ci == 0 or need > CHUNK_PIECE[ci - 1]:
                            tensor.wait_ge(s_x[p], 16 * need)
                        if c >= 4:
                            # WAR: bank slot c%4 last used by chunk c-4
                            tensor.wait_ge(s_cp, c - 3)
                            tensor.wait_ge(s_cp2, c - 3)
                        # the final chunk is split into two 4-row sub-chunks so
                        # its first copy+output DMA overlap the last matmuls;
                        # sub-chunk B uses the next bank slot at offset 0 (PSUM
                        # accumulation must start at a bank base)
                        for r0, r1 in SUBCHUNKS[c]:
                            slot = c % 4 if r0 == 0 else (c + 1) % 4
                            if r0 != 0:
                                tensor.wait_ge(s_cp, c - 2)
                                tensor.wait_ge(s_cp2, c - 2)
                            for k in range(KS * KS):
                                di, dj = divmod(k, KS)
                                last = k == KS * KS - 1
                                for half in range(2):
                                    c0 = half * C_IN
                                    mm = tensor.matmul(
                                        out=banks[slot][half][:, 0 : r1 - r0, :],
                                        lhsT=wt[c0 : c0 + C_IN, k, :],
                                        rhs=xps[p][
                                            c0 : c0 + C_IN,
                                            h0 + di + r0 : h0 + di + r1,
                                            dj : dj + H,
                                        ],
                                        start=(k == 0),
                                        stop=last,
                                    )
                                    if last and half == 1:
                                        mm.then_inc(s_mm, 1)

            @block.vector
            def _(vector: bass.BassEngine):
                for p in range(2):
                    for ci in range(NCHUNK):
                        c = p * NCHUNK + ci
                        h0 = ci * RCHUNK
                        blo, bhi = next(b for b in OBLOCKS if b[0] <= h0 < b[1])
                        bi = OBLOCKS.index((blo, bhi))
                        for r0, r1 in SUBCHUNKS[c]:
                            slot = c % 4 if r0 == 0 else (c + 1) % 4
                            vector.wait_ge(s_mm, SUBIDX[(c, r0)])
                            vector.tensor_copy(
                                out=obs[p][0][bi][
                                    :, h0 - blo + r0 : h0 - blo + r1, :
                                ],
                                in_=banks[slot][0][:, 0 : r1 - r0, :],
                            ).then_inc(s_cp, 1)

    return nc


def _prep_inputs(x, K, mm_dt=MM_DT):
    np_dt = mybir.dt.np(mm_dt)
    x = np.ascontiguousarray(np.asarray(x, dtype=np.float32))
    K = np.ascontiguousarray(np.asarray(K, dtype=np.float32))
    xpad = np.pad(x, ((0, 0), (0, 0), (1, 1), (1, 1))).astype(np_dt)
    Wt = K.reshape(KS * KS * C_IN, C_OUT).reshape(C_IN, KS * KS, C_OUT)
    Wrep = np.ascontiguousarray(np.concatenate([Wt, Wt], axis=0)).astype(np_dt)
    shards = xpad.reshape(N_CORES, BPC, C_IN, HP, HP)
    return [{"x": np.ascontiguousarray(shards[i]), "w": Wrep} for i in range(N_CORES)]


def run(x, K, trace=False, mm_dt=MM_DT):
    nc = build_nc(mm_dt)
    in_maps = _prep_inputs(x, K, mm_dt)
    res = run_bass_kernel_spmd(nc, in_maps, list(range(N_CORES)), trace=trace)
    out = np.concatenate(
        [res.results[i]["out"] for i in range(N_CORES)], axis=0
    ).astype(np.float32)
    return out, res


def kernel(x, K):
    out, _ = run(x, K, trace=False)
    return out


# revision 12
# speedup vs baseline: 1.0312x; 1.0312x over previous
"""Trainium2 Bass kernel for nn_Conv2d_24833500905755 (3x3 conv, B=32,
C_in=64, C_out=128, 56x56, pad 1, with the reference's mismatched
weight-flatten order).

Math: out[b,co,h,w] = sum_{c,di,dj} xpad[b,c,h+di,w+dj] * Wt[c,di*3+dj,co]
with Wt = K.reshape(576, C_OUT).reshape(C_IN, 9, C_OUT).

Data-parallel: 4 images per NeuronCore, 2 images packed on the
128-partition dim (fp16 matmuls, K=64 contraction per half, concurrent
PE row-group tiles). Raw-bass hand-scheduled engine programs with
manual semaphores.

DMA plan: the three DMA paths (gpsimd SWDGE ring, Sync HWDGE ring,
Scalar HWDGE ring, each in-order at ~160-190GB/s, ~1.7-2.5us startup)
are specialized so the first-chunk gates (w and x rows 0-10) land as
early as possible:
  gpsimd ring: w (294KB) only                       -> ~9.3us
  Sync ring:   x pair-0 pieces, then half-0 outputs -> x0A ~9.6us
  Scalar ring: x pair-1 pieces, then half-1 outputs
Tensor: 11 junk warmup matmuls (HAM p-state ramp + DMA-latency cover -
any tensor idle gap resets the PE clock ramp and costs ~3us of
half-clock stream, so warmups must bridge seamlessly into the real
stream), then 252 fp16 matmuls gated on input-piece / bank-WAR sems.
Vector: 14 PSUM->SBUF copies (half 0); Scalar: 14 copies (half 1).

Outputs are staged and DMA'd as fp16 (halves HBM write traffic vs
fp32); the host casts back to fp32. PSUM accumulation stays fp32, so
this only adds ~2^-11 relative rounding on the final values.
"""

from contextlib import ExitStack

import numpy as np

import concourse.bass as bass
import concourse.mybir as mybir
from concourse.bass_utils import run_bass_kernel_spmd

B, C_IN, C_OUT, H = 32, 64, 128, 56
KS = 3
N_CORES = 8
BPC = B // N_CORES
HP = H + 2
RCHUNK = 8
NCHUNK = H // RCHUNK          # 7 chunks/image, 14 global chunks (2 pairs)
OBLOCKS = [(0, 24), (24, 40), (40, 48), (48, 56)]
# x row pieces: piece j covers rows [XSPLITS[j], XSPLITS[j+1]).
# chunk ci needs rows [8ci, 8ci+9]: c0<-p0, c1,c2<-p1, c3,c4<-p2, c5,c6<-p3
XSPLITS = [0, 11, 25, 42, HP]
# chunk index -> number of pieces that must have landed
CHUNK_PIECE = [1, 2, 2, 3, 3, 4, 4]
MM_DT = mybir.dt.float16
N_WARMUP = 19
# per-global-chunk row sub-ranges; the very last chunk is split so its
# first copy + output DMA overlap the final matmuls
SUBCHUNKS = {c: [(0, RCHUNK)] for c in range(2 * NCHUNK)}
SUBCHUNKS[2 * NCHUNK - 1] = [(0, 4), (4, RCHUNK)]
# (chunk, sub_r0) -> 1-based completion index in the s_mm/s_cp counting
SUBIDX = {}
_n = 0
for _c in range(2 * NCHUNK):
    for _r0, _r1 in SUBCHUNKS[_c]:
        _n += 1
        SUBIDX[(_c, _r0)] = _n


def build_nc(mm_dt=MM_DT):
    nc = bass.Bass()
    x_ext = nc.declare_dram_parameter("x", [BPC, C_IN, HP, HP], mm_dt, isOutput=False)
    w_ext = nc.declare_dram_parameter("w", [2 * C_IN, KS * KS, C_OUT], mm_dt, isOutput=False)
    out_ext = nc.declare_dram_parameter("out", [BPC, C_OUT, H, H], mm_dt, isOutput=True)

    n_out_dmas = 2 * len(OBLOCKS) * 2  # pairs * blocks * halves

    with ExitStack() as ctx:
        wt = ctx.enter_context(nc.sbuf_tensor("wt", [2 * C_IN, KS * KS, C_OUT], mm_dt))
        xps = [
            ctx.enter_context(nc.sbuf_tensor(f"xp{p}", [2 * C_IN, HP, HP], mm_dt))
            for p in range(2)
        ]
        # obs[p][half][block]
        obs = [
            [
                [
                    ctx.enter_context(
                        nc.sbuf_tensor(f"ob_{p}_{h}_{bi}", [C_OUT, bhi - blo, H], mm_dt)
                    )
                    for bi, (blo, bhi) in enumerate(OBLOCKS)
                ]
                for h in range(2)
            ]
            for p in range(2)
        ]
        # banks[slot][half] - 8 PSUM banks
        banks = [
            [
                ctx.enter_context(
                    nc.psum_tensor(f"ps_{s}_{h}", [C_OUT, RCHUNK, H], mybir.dt.float32)
                )
                for h in range(2)
            ]
            for s in range(4)
        ]
        s_w = ctx.enter_context(nc.semaphore("s_w"))
        s_x = [ctx.enter_context(nc.semaphore(f"s_x{p}")) for p in range(2)]
        s_mm = ctx.enter_context(nc.semaphore("s_mm"))
        s_cp = ctx.enter_context(nc.semaphore("s_cp"))
        s_cp2 = ctx.enter_context(nc.semaphore("s_cp2"))
        s_out = ctx.enter_context(nc.semaphore("s_out"))

        with nc.Block() as block:

            @block.sync
            def _(sync: bass.BassEngine):
                src = x_ext[0:2].rearrange("b c h w -> (b c) h w")
                for j in range(len(XSPLITS) - 1):
                    lo, hi = XSPLITS[j], XSPLITS[j + 1]
                    sync.dma_start(
                        out=xps[0][:, lo:hi, :], in_=src[:, lo:hi, :]
                    ).then_inc(s_x[0], 16)
                for p in range(2):
                    for bi, (blo, bhi) in enumerate(OBLOCKS):
                        c_last = p * NCHUNK + (bhi // RCHUNK - 1)
                        dst = out_ext[2 * p : 2 * p + 1].rearrange("b c h w -> (b c) h w")
                        for r0, r1 in SUBCHUNKS[c_last]:
                            h0 = (bhi // RCHUNK - 1) * RCHUNK
                            sync.wait_ge(s_cp, SUBIDX[(c_last, r0)])
                            dlo = blo if len(SUBCHUNKS[c_last]) == 1 else h0 + r0
                            sync.dma_start(
                                out=dst[:, dlo : h0 + r1, :],
                                in_=obs[p][0][bi][:, dlo - blo : h0 + r1 - blo, :],
                            ).then_inc(s_out, 16)

            @block.scalar
            def _(scalar: bass.BassEngine):
                scalar.dma_start(out=wt[:], in_=w_ext[:]).then_inc(s_w, 16)
                src = x_ext[2:4].rearrange("b c h w -> (b c) h w")
                for j in range(len(XSPLITS) - 1):
                    lo, hi = XSPLITS[j], XSPLITS[j + 1]
                    scalar.dma_start(
                        out=xps[1][:, lo:hi, :], in_=src[:, lo:hi, :]
                    ).then_inc(s_x[1], 16)
                for p in range(2):
                    for ci in range(NCHUNK):
                        c = p * NCHUNK + ci
                        h0 = ci * RCHUNK
                        blo, bhi = next(b for b in OBLOCKS if b[0] <= h0 < b[1])
                        bi = OBLOCKS.index((blo, bhi))
                        for r0, r1 in SUBCHUNKS[c]:
                            slot = c % 4 if r0 == 0 else (c + 1) % 4
                            scalar.wait_ge(s_mm, SUBIDX[(c, r0)])
                            scalar.copy(
                                out=obs[p][1][bi][
                                    :, h0 - blo + r0 : h0 - blo + r1, :
                                ],
                                in_=banks[slot][1][:, 0 : r1 - r0, :],
                            ).then_inc(s_cp2, 1)
                        dst = out_ext[2 * p + 1 : 2 * p + 2].rearrange(
                            "b c h w -> (b c) h w"
                        )
                        if len(SUBCHUNKS[c]) > 1:
                            for r0, r1 in SUBCHUNKS[c]:
                                scalar.dma_start(
                                    out=dst[:, h0 + r0 : h0 + r1, :],
                                    in_=obs[p][1][bi][
                                        :, h0 + r0 - blo : h0 + r1 - blo, :
                                    ],
                                ).then_inc(s_out, 16)
                        elif h0 + RCHUNK == bhi:
                            scalar.wait_ge(s_cp2, SUBIDX[(c, 0)])
                            scalar.dma_start(
                                out=dst[:, blo:bhi, :], in_=obs[p][1][bi][:]
                            ).then_inc(s_out, 16)

            @block.tensor
            def _(tensor: bass.BassEngine):
                # HAM warm-up: junk matmuls on not-yet-loaded SBUF while the
                # first input DMAs land; keeps the PE ramping toward 8/8
                # clock for the real stream. banks[3] is first reused by
                # chunk 3 (start=True clears it), well after these complete.
                for wi in range(N_WARMUP):
                    h = wi % 2
                    c0 = h * C_IN
                    tensor.matmul(
                        out=banks[3][h][:],
                        lhsT=wt[c0 : c0 + C_IN, 0, :],
                        rhs=xps[0][c0 : c0 + C_IN, 0:RCHUNK, 0:H],
                        start=True,
                        stop=True,
                    )
                tensor.wait_ge(s_w, 16)
                for p in range(2):
                    for ci in range(NCHUNK):
                        c = p * NCHUNK + ci
                        h0 = ci * RCHUNK
                        need = CHUNK_PIECE[ci]
                        if ci == 0 or need > CHUNK_PIECE[ci - 1]:
                            tensor.wait_ge(s_x[p], 16 * need)
                        if c >= 4:
                            # WAR: bank slot c%4 last used by chunk c-4
                            tensor.wait_ge(s_cp, c - 3)
                            tensor.wait_ge(s_cp2, c - 3)
                        # the final chunk is split into two 4-row sub-chunks so
                        # its first copy+output DMA overlap the last matmuls;
                        # sub-chunk B uses the next bank slot at offset 0 (PSUM
                        # accumulation must start at a bank base)
                        for r0, r1 in SUBCHUNKS[c]:
                            slot = c % 4 if r0 == 0 else (c + 1) % 4
                            if r0 != 0:
                                tensor.wait_ge(s_cp, c - 2)
                                tensor.wait_ge(s_cp2, c - 2)
                            for k in range(KS * KS):
                                di, dj = divmod(k, KS)
                                last = k == KS * KS - 1
                                for half in range(2):
                                    c0 = half * C_IN
                                    mm = tensor.matmul(
                                        out=banks[slot][half][:, 0 : r1 - r0, :],
                                        lhsT=wt[c0 : c0 + C_IN, k, :],
                                        rhs=xps[p][
                                            c0 : c0 + C_IN,
                                            h0 + di + r0 : h0 + di + r1,
                                            dj : dj + H,
                                        ],
                                        start=(k == 0),
                                        stop=last,
                                    )
                                    if last and half == 1:
                                        mm.then_inc(s_mm, 1)

            @block.vector
            def _(vector: bass.BassEngine):
                for p in range(2):
                    for ci in range(NCHUNK):
                        c = p * NCHUNK + ci
                        h0 = ci * RCHUNK
                        blo, bhi = next(b for b in OBLOCKS if b[0] <= h0 < b[1])
                        bi = OBLOCKS.index((blo, bhi))
                        for r0, r1 in SUBCHUNKS[c]:
                            slot = c % 4 if r0 == 0 else (c + 1) % 4
                            vector.wait_ge(s_mm, SUBIDX[(c, r0)])
                            vector.tensor_copy(
                                out=obs[p][0][bi][
                                    :, h0 - blo + r0 : h0 - blo + r1, :
                                ],
                                in_=banks[slot][0][:, 0 : r1 - r0, :],
                            ).then_inc(s_cp, 1)

    return nc


def _prep_inputs(x, K, mm_dt=MM_DT):
    np_dt = mybir.dt.np(mm_dt)
    x = np.ascontiguousarray(np.asarray(x, dtype=np.float32))
    K = np.ascontiguousarray(np.asarray(K, dtype=np.float32))
    xpad = np.pad(x, ((0, 0), (0, 0), (1, 1), (1, 1))).astype(np_dt)
    Wt = K.reshape(KS * KS * C_IN, C_OUT).reshape(C_IN, KS * KS, C_OUT)
    Wrep = np.ascontiguousarray(np.concatenate([Wt, Wt], axis=0)).astype(np_dt)
    shards = xpad.reshape(N_CORES, BPC, C_IN, HP, HP)
    return [{"x": np.ascontiguousarray(shards[i]), "w": Wrep} for i in range(N_CORES)]


def run(x, K, trace=False, mm_dt=MM_DT):
    nc = build_nc(mm_dt)
    in_maps = _prep_inputs(x, K, mm_dt)
    res = run_bass_kernel_spmd(nc, in_maps, list(range(N_CORES)), trace=trace)
    out = np.concatenate(
        [res.results[i]["out"] for i in range(N_CORES)], axis=0
    ).astype(np.float32)
    return out, res


def kernel(x, K):
    out, _ = run(x, K, trace=False)
    return out
